# revision 54
# baseline (speedup 1.0000x reference)
"""VQ codebook encoding kernel for Trainium2 (8 NeuronCores, data-parallel over batch).

Per batch b (tokens n = H*W flattened, D features, K codes):
  dist[n,k] = s2[k] * (||x[n]||^2 - 2 x[n].codes[k] + ||codes[k]||^2)
  a = softmax_k(dist)
  e[b,k,d] = sum_n a[n,k] * x[n,d] - (sum_n a[n,k]) * codes[k,d]

Softmax shift: dist*[n,k] = dist[n,k] - s2max * x2[n] subtracts a k-independent
term, so softmax is unchanged and dist* <= ~1.3 — no per-token max pass needed
(the argmax-s2 code keeps its exponent near 0, so the denominator never
underflows).

Interleaved-chunk layout: per supertile of 2048 tokens, mm1 strip g takes the
32-token blocks B with B % 4 == g (a 2-D strided moving operand), packing the 4
strips into one (128, 512) PSUM bank via PE column strips. A single DVE
StreamTranspose (32x32 blocks transposed in place) then yields dist for token
t = 128*m + 32*g + q at partition 32*g+q of f-tile m — i.e. plain consecutive
128-token tiles, so the x transposes use contiguous stationary slices and mm2
needs no extra data movement. This replaces 32 PE dist-transposes + PSUM
copies per supertile. Weight (stationary) APs must be 1-D free on real HW;
gpsimd cannot touch PSUM or run 3-operand TensorScalarPtr.

Sharding: batch B=16 split across 8 cores (2 per core); codes/scale replicated.
"""

import sys

sys.path.insert(0, "/opt/trn_rl_repo")
import numpy as np

import concourse.bass as bass
import concourse.bacc as bacc
import concourse.tile as tile
from concourse import mybir
from concourse.masks import make_identity

FP32 = mybir.dt.float32
BF16 = mybir.dt.bfloat16
AF = mybir.ActivationFunctionType
ALU = mybir.AluOpType
AX = mybir.AxisListType

K = 32
P = 128

B_FULL, D_FULL, H_FULL, W_FULL = 16, 512, 64, 64
N_FULL = H_FULL * W_FULL
NCORES = 8
BS = B_FULL // NCORES

ST = 2048  # tokens per supertile
NST = N_FULL // ST  # supertiles per batch
NF = ST // P  # 16 f-blocks (of 128 gathered tokens) per supertile
G = 4  # chunk groups per supertile
DT = D_FULL // P  # 4 d-tiles

# engine rotation: psx -> xt pair-copies (8 per supertile) and x2 ops (16)
COPY_ENG = ["v", "v", "a", "v"]
X2_ENG = ["a", "v", "a", "v", "a", "v", "v", "a", "v", "a", "v", "a", "a", "v", "a", "v"]
# last supertile: keep the slow Pool engine off the critical tail
COPY_ENG_LAST = ["a", "v", "a", "v", "a", "v", "a", "a"]
X2_ENG_LAST = ["v", "a"] * 8


def build(nc, bs=BS, d=D_FULL, n=N_FULL):
    x_d = nc.dram_tensor("x", (bs, DT, P, n), FP32, kind="ExternalInput").ap()
    codes_d = nc.dram_tensor("codes", (K, d), FP32, kind="ExternalInput").ap()
    scale_d = nc.dram_tensor("scale", (K, 1), FP32, kind="ExternalInput").ap()
    e_d = nc.dram_tensor("e", (bs, K, d), FP32, kind="ExternalOutput").ap()

    with tile.TileContext(nc) as tc:
        with (
            tc.tile_pool(name="const", bufs=1) as constp,
            tc.tile_pool(name="xb", bufs=3) as xbp,
            tc.tile_pool(name="xt", bufs=3) as xtp,
            tc.tile_pool(name="dist", bufs=2) as distp,
            tc.tile_pool(name="smax", bufs=2) as smaxp,
            tc.tile_pool(name="misc", bufs=4) as miscp,
            tc.tile_pool(name="tail", bufs=2) as tailp,
            tc.tile_pool(name="ps_sup", bufs=2, space="PSUM") as ps_supp,
            tc.tile_pool(name="ps_x", bufs=2, space="PSUM") as psxp,
            tc.tile_pool(name="ps_e", bufs=1, space="PSUM") as ps_ep,
            tc.tile_pool(name="ps_small", bufs=1, space="PSUM") as ps_smallp,
            tc.tile_pool(name="dstage", bufs=1, space="DRAM") as dstagep,
        ):
            # ---------------- one-time constants ----------------
            codes_sb = constp.tile([K, d], FP32)
            nc.sync.dma_start(out=codes_sb, in_=codes_d)
            scale_col = constp.tile([K, 1], FP32)
            nc.sync.dma_start(out=scale_col, in_=scale_d)
            scale_row = constp.tile([1, K], FP32)
            nc.sync.dma_start(out=scale_row, in_=scale_d.rearrange("a b -> b a"))

            ident_bf = constp.tile([P, P], BF16)
            make_identity(nc, ident_bf)

            s2_col = constp.tile([K, 1], FP32)
            nc.vector.tensor_mul(s2_col, scale_col, scale_col)
            s2_row = constp.tile([1, K], FP32)
            nc.vector.tensor_mul(s2_row, scale_row, scale_row)

            sq_codes = constp.tile([K, d], FP32)
            c2_col = constp.tile([K, 1], FP32)
            nc.scalar.activation(
                out=sq_codes, in_=codes_sb, func=AF.Square, accum_out=c2_col
            )
            # c2 as a row via a DRAM round-trip (SBUF APs cannot cross partitions)
            stage_c2 = dstagep.tile([K, 1], FP32)
            nc.sync.dma_start(out=stage_c2, in_=c2_col)
            c2_row = constp.tile([1, K], FP32)
            nc.sync.dma_start(out=c2_row, in_=stage_c2[:].rearrange("a b -> b a"))
            neg2s2_col = constp.tile([K, 1], FP32)
            nc.vector.tensor_scalar_mul(neg2s2_col, s2_col, -2.0)

            # mm1 weights w[k, d] = -2 * s2[k] * codes[k, d], transposed into
            # (P, DT*K) column blocks, bf16
            w_kn = constp.tile([K, d], BF16)
            nc.vector.tensor_scalar_mul(w_kn, codes_sb, neg2s2_col)
            w_mm1 = constp.tile([P, DT * K], BF16)
            for j in range(DT):
                psw = psxp.tile([P, 4, 512], BF16, tag="psx")
                nc.tensor.transpose(
                    psw[:, 0, :K], w_kn[:, j * P : (j + 1) * P], ident_bf[:K, :K]
                )
                nc.vector.tensor_copy(w_mm1[:, j * K : (j + 1) * K], psw[:, 0, :K])

            # s2c2 as a (1, G*K) bf16 row: the mm1 init matmul adds it to every
            # column strip while seeding the bank's has_written bits
            s2c2_row = constp.tile([1, 1, K], FP32)
            nc.vector.tensor_mul(s2c2_row, s2_row, c2_row)
            s2c2_pack = constp.tile([1, G, K], BF16)

            # s2m = s2 - max(s2), broadcast to all partitions with k on free
            s2max = constp.tile([1, 1], FP32)
            nc.vector.reduce_max(s2max, s2_row, axis=AX.X)
            s2m_row = constp.tile([1, K], FP32)
            nc.vector.tensor_scalar_sub(s2m_row, s2_row, s2max)
            stage_s2m = dstagep.tile([1, 1, K], FP32)
            nc.sync.dma_start(out=stage_s2m, in_=s2m_row)
            s2m_bc = constp.tile([P, 1, K], FP32)
            nc.sync.dma_start(out=s2m_bc, in_=stage_s2m[:].to_broadcast([P, 1, K]))

            ones_row = constp.tile([1, NF, K], BF16)
            nc.vector.memset(ones_row, 1.0)
            ones_col_f32 = constp.tile([P, 1], FP32)
            nc.vector.memset(ones_col_f32, 1.0)
            zeros_row = constp.tile([1, P], BF16)
            nc.vector.memset(zeros_row, 0.0)

            # ---------------- x loads: SWDGE cast-DMAs, finer for early tiles --
            sts = [(b, sg) for b in range(bs) for sg in range(NST)]
            xb_all = []
            for si, (b, sg) in enumerate(sts):
                xb = xbp.tile([P, DT, NF, G, K], BF16, tag="xb", name="xb")
                xb_all.append(xb)

            def emit_load(si, parts):
                b, sg = sts[si]
                mstep = NF // parts
                for gp in range(parts):
                    m0 = gp * mstep
                    t0 = sg * ST + m0 * P
                    nc.gpsimd.dma_start(
                        out=xb_all[si][:, :, m0 : m0 + mstep, :, :, ],
                        in_=x_d[b, :, :, t0 : t0 + mstep * P].transpose([1, 0, 2]),
                    )

            emit_load(0, 4)
            # issued between load gens: late enough that its input is ready,
            # early enough for supertile 0 (only gpsimd DMAs can cast)
            nc.gpsimd.dma_start(
                out=s2c2_pack, in_=s2c2_row[:].to_broadcast([1, G, K])
            )
            emit_load(1, 4)

            # ---------------- supertile pipeline ----------------
            xt_all, a_all, asum_all, x2_all, dist_all = {}, {}, {}, {}, {}
            ps_e_all, ps_cs_all = {}, {}

            def emit_softmax_part(si, nparts, h):
                nf2 = NF // nparts
                sl = slice(h * nf2, (h + 1) * nf2)
                eng = nc.vector if si == nsts - 1 else nc.gpsimd
                m1 = smaxp.tile([P, nf2, K], FP32, tag=f"m1{h % 2}")
                eng.tensor_mul(
                    m1,
                    s2m_bc[:].to_broadcast([P, nf2, K]),
                    x2_all[si][:, sl, :].to_broadcast([P, nf2, K]),
                )
                eng.tensor_add(m1, m1, dist_all[si][:, sl, :])
                pexp = smaxp.tile([P, nf2, K], FP32, tag=f"pexp{h % 2}")
                nc.scalar.activation(pexp, m1, AF.Exp)
                scol = smaxp.tile([P, nf2, 1], FP32, tag=f"scol{h % 2}")
                nc.vector.reduce_sum(scol, pexp, axis=AX.X)
                rcol = smaxp.tile([P, nf2, 1], FP32, tag=f"rcol{h % 2}")
                nc.vector.reciprocal(rcol, scol)
                nc.gpsimd.tensor_mul(
                    a_all[si][:, sl, :], pexp, rcol[:].to_broadcast([P, nf2, K])
                )

            def emit_asum(si, nparts, h):
                # f-reduction of a for the colsum matmul (fp32 accumulate)
                nf2 = NF // nparts
                sl = slice(h * nf2, (h + 1) * nf2)
                nc.vector.tensor_reduce(
                    out=asum_all[si][:, :, h : h + 1],
                    in_=a_all[si][:, sl, :].transpose([0, 2, 1]),
                    axis=AX.X,
                    op=ALU.add,
                )

            def emit_mm2_pair(pi, fp):
                pb, psg = sts[pi]
                first = psg == 0 and fp == 0
                if first:
                    ps_e_all[pb] = ps_ep.tile([K, 512], FP32, tag="e", name="pse")
                for f in (2 * fp, 2 * fp + 1):
                    nc.tensor.matmul(
                        ps_e_all[pb],
                        a_all[pi][:, f, :],
                        xt_all[pi][:, f, :],
                        start=(first and f == 0),
                        stop=(psg == NST - 1 and f == NF - 1),
                        skip_group_check=True,
                    )

            def emit_cs(pi):
                pb, psg = sts[pi]
                nparts = nparts_all[pi]
                if psg == 0:
                    ps_cs_all[pb] = ps_smallp.tile([K, 1], FP32, tag="cs", name="pscs")
                for h in range(nparts):
                    nc.tensor.matmul(
                        ps_cs_all[pb],
                        asum_all[pi][:, :, h : h + 1],
                        ones_col_f32,
                        start=(psg == 0 and h == 0),
                        stop=(psg == NST - 1 and h == nparts - 1),
                        skip_group_check=True,
                    )

            negcs_all = {}

            def emit_negcs(pi):
                pb, psg = sts[pi]
                if psg == NST - 1:
                    negcs_all[pb] = tailp.tile([K, 1], FP32, tag="negcs", name="negcs")
                    nc.scalar.mul(negcs_all[pb], ps_cs_all[pb], -1.0)

            def emit_tail(pi):
                pb, psg = sts[pi]
                if psg == NST - 1:
                    # tail: e = e1 - cs * codes, read straight out of PSUM
                    negcs = negcs_all[pb]
                    e_sb = tailp.tile([K, d], FP32, tag="esb")
                    nc.vector.scalar_tensor_tensor(
                        out=e_sb,
                        in0=codes_sb,
                        scalar=negcs,
                        in1=ps_e_all[pb],
                        op0=ALU.mult,
                        op1=ALU.add,
                    )
                    nc.sync.dma_start(out=e_d[pb], in_=e_sb)

            def emit_mm1(si, ghalf):
                """Half of si's mm1 (column groups 2*ghalf, 2*ghalf+1)."""
                xb = xb_all[si]
                if ghalf == 0:
                    ps_sup = ps_supp.tile([P, NF, K], FP32, tag="sup", name="sup")
                    ps_sup_all[si] = ps_sup
                    nc.tensor.matmul(
                        ps_sup,
                        zeros_row,
                        ones_row,
                        start=True,
                        stop=False,
                        skip_group_check=True,
                    )
                mranges = [(m, m + 4) for m in range(0, NF, 4)]
                for m0, m1 in mranges:
                    for g in (2 * ghalf, 2 * ghalf + 1):
                        for j in range(DT):
                            nc.tensor.matmul(
                                ps_sup_all[si][32 * g : 32 * g + 32, m0:m1, :],
                                w_mm1[:, j * K : (j + 1) * K],
                                xb[:, j, m0:m1, g, :],
                                start=False,
                                stop=False,
                                tile_position=(0, 32 * g),
                                skip_group_check=True,
                            )
                if ghalf == 1 and si > 0:
                    emit_s2c2_edge(si)

            def emit_s2c2_edge(si):
                # s2c2 bias for every strip; kept off the critical setup
                # path by running it last (also carries the stop)
                nc.tensor.matmul(
                    ps_sup_all[si],
                    s2c2_pack,
                    ones_row,
                    start=False,
                    stop=True,
                    skip_group_check=True,
                )

            def emit_x2(si, f):
                x2e = (X2_ENG_LAST if si == nsts - 1 else X2_ENG)[f]
                src = xt_all[si][:, f, :]
                sqs = miscp.tile([P, 512], BF16, tag="sqs")
                if x2e == "a":
                    nc.scalar.activation(
                        out=sqs,
                        in_=src,
                        func=AF.Square,
                        accum_out=x2_all[si][:, f, :],
                    )
                else:
                    nc.vector.scalar_tensor_tensor(
                        out=sqs,
                        in0=src,
                        scalar=1.0,
                        in1=src,
                        op0=ALU.mult,
                        op1=ALU.mult,
                        accum_out=x2_all[si][:, f, :],
                    )

            ps_sup_all = {}
            nparts_all = {}
            nsts = len(sts)

            def emit_prepare_next(pi):
                fp = prep_ctr[pi]
                if fp >= NF // 2:
                    return
                if fp == 0:
                    emit_prepare_alloc(pi)
                prep_ctr[pi] += 1
                emit_prepare_fp(pi, fp)

            cur_psx = {}

            def emit_prepare_fp(pi, fp):
                """Transposes into a quad psx; copy once per quad (2 f-pairs),
                x2 lagged one quad so it never delays a copy."""
                xb = xb_all[pi]
                if fp % 2 == 0:
                    cur_psx[pi] = psxp.tile([P, 4, 512], BF16, tag="psx", name="psx")
                psx = cur_psx[pi]
                for tt in range(2):
                    f = 2 * fp + tt
                    for j in range(DT):
                        nc.tensor.transpose(
                            psx[:, 2 * (fp % 2) + tt, j * P : (j + 1) * P],
                            xb[:, j, f, :, :],
                            ident_bf,
                        )
                if fp % 2 == 1:
                    ce = COPY_ENG[fp // 2]
                    xt_quad = xt_all[pi][:, 2 * fp - 2 : 2 * fp + 2, :]
                    if ce == "v":
                        nc.vector.tensor_copy(xt_quad, psx)
                    else:
                        nc.scalar.copy(xt_quad, psx)
                elif fp >= 2:
                    # x2 for the previous quad's four f-tiles
                    for f in range(2 * fp - 4, 2 * fp):
                        emit_x2(pi, f)

            def emit_prepare_alloc(pi):
                xt_all[pi] = xtp.tile([P, NF, 512], BF16, tag="xt", name="xt")
                x2_all[pi] = miscp.tile([P, NF, 1], FP32, tag="x2", name="x2")
                a_all[pi] = smaxp.tile([P, NF, K], BF16, tag="a", name="a")
                nparts_all[pi] = 2
                asum_all[pi] = smaxp.tile([P, K, 2], FP32, tag="asum", name="asum")

            def emit_streamT(pi):
                dist = distp.tile([P, NF, K], FP32, tag="dist")
                dist_all[pi] = dist
                nc.vector.transpose(dist, ps_sup_all[pi])

            pair_ctr = {si: 0 for si in range(nsts)}
            prep_ctr = {si: 0 for si in range(nsts)}
            x2_done = {si: False for si in range(nsts)}
            sm_done = {si: 0 for si in range(nsts)}  # softmax halves emitted

            def emit_mm2_n(pi, n):
                for _ in range(n):
                    fp = pair_ctr[pi]
                    if fp >= NF // 2:
                        return
                    pair_ctr[pi] += 1
                    emit_mm2_pair(pi, fp)

            def emit_sm(si):
                emit_softmax_part(si, 2, sm_done[si])
                sm_done[si] += 1

            # ---------------- prologue: fully prepare supertile 0 ------------
            emit_mm1(0, 0)
            emit_mm1(0, 1)
            for fp in range(NF // 2):
                emit_prepare_next(0)
                if fp == 3:
                    emit_mm1(1, 0)
                elif fp == 4:
                    emit_s2c2_edge(0)
                    emit_streamT(0)
                elif fp == 5:
                    emit_sm(0)
            emit_mm1(1, 1)
            x2_done[0] = True
            for f in range(NF - 4, NF):
                emit_x2(0, f)
            emit_mm2_n(0, 2)

            # ---------------- main blocks: softmax+mm2(si), prepare(si+1) ----
            for si in range(nsts):
                nxt = si + 1 if si + 1 < nsts else None
                same_batch_nxt = nxt is not None and sts[nxt][0] == sts[si][0]
                while sm_done[si] < 2 and (sm_done[si] == 0 or si == nsts - 1):
                    emit_sm(si)
                pre_done = sm_done[si] == 2 and si == nsts - 1
                if pre_done:
                    emit_asum(si, 2, 0)
                    emit_asum(si, 2, 1)
                if nxt is not None:
                    emit_streamT(nxt)
                for fp in range(NF // 2):
                    if nxt is not None:
                        emit_prepare_next(nxt)
                        if prep_ctr[nxt] == NF // 2 and not x2_done[nxt]:
                            x2_done[nxt] = True
                            for f in range(NF - 4, NF):
                                emit_x2(nxt, f)
                            if same_batch_nxt and nxt == nsts - 1:
                                emit_sm(nxt)
                    emit_mm2_n(si, 1)
                    if fp == 1 and si + 2 < nsts:
                        emit_load(si + 2, 4)
                    if fp == 1 and sm_done[si] < 2:
                        emit_sm(si)
                    if fp == 2 and not pre_done:
                        emit_asum(si, 2, 0)
                    if fp == 2 and pre_done:
                        emit_cs(si)
                        emit_negcs(si)
                    if fp == 5:
                        if sm_done[si] == 2 and not pre_done:
                            emit_asum(si, 2, 1)
                        if si + 2 < nsts:
                            emit_mm1(si + 2, 0)
                        if same_batch_nxt:
                            emit_sm(nxt)
                    if fp == 6 and sm_done[si] == 2 and not pre_done:
                        emit_cs(si)
                        emit_negcs(si)
                    elif fp == 7 and si + 2 < nsts:
                        emit_mm1(si + 2, 1)
                if si + 2 == nsts - 1:
                    emit_prepare_next(nsts - 1)
                    emit_prepare_next(nsts - 1)
                if nxt is not None and not x2_done[nxt] and prep_ctr[nxt] == NF // 2:
                    x2_done[nxt] = True
                    for f in range(NF - 4, NF):
                        emit_x2(nxt, f)
                    if same_batch_nxt and nxt == nsts - 1:
                        emit_sm(nxt)
                emit_mm2_n(si, NF // 2)  # any pairs not yet emitted
                if same_batch_nxt:
                    emit_mm2_n(nxt, 2)
                emit_tail(si)


_CACHE = {}


def _get_compiled():
    if "nc" not in _CACHE:
        nc = bacc.Bacc(
            "TRN2",
            target_bir_lowering=False,
            debug=False,
            dynamic_dma_scratch_size=98304,
        )
        build(nc)
        nc.compile()
        _CACHE["nc"] = nc
    return _CACHE["nc"]


def kernel(x, codes, scale):
    from concourse import bass_utils

    b_total = x.shape[0]
    bs = b_total // NCORES
    xr = np.ascontiguousarray(x.reshape(b_total, DT, P, -1), dtype=np.float32)
    codes_c = np.ascontiguousarray(codes, dtype=np.float32)
    scale_c = np.ascontiguousarray(scale, dtype=np.float32).reshape(K, 1)

    nc = _get_compiled()
    in_maps = [
        {"x": xr[i * bs : (i + 1) * bs], "codes": codes_c, "scale": scale_c}
        for i in range(NCORES)
    ]
    res = bass_utils.run_bass_kernel_spmd(nc, in_maps, core_ids=list(range(NCORES)))
    e = np.concatenate([r["e"] for r in res.results], axis=0)
    return e.astype(np.float32)


# revision 55
# speedup vs baseline: 1.0732x; 1.0732x over previous
"""VQ codebook encoding kernel for Trainium2 (8 NeuronCores, data-parallel over batch).

Per batch b (tokens n = H*W flattened, D features, K codes):
  dist[n,k] = s2[k] * (||x[n]||^2 - 2 x[n].codes[k] + ||codes[k]||^2)
  a = softmax_k(dist)
  e[b,k,d] = sum_n a[n,k] * x[n,d] - (sum_n a[n,k]) * codes[k,d]

Softmax shift: dist*[n,k] = dist[n,k] - s2max * x2[n] subtracts a k-independent
term, so softmax is unchanged and dist* <= ~1.3 — no per-token max pass needed
(the argmax-s2 code keeps its exponent near 0, so the denominator never
underflows).

Interleaved-chunk layout: per supertile of 2048 tokens, mm1 strip g takes the
32-token blocks B with B % 4 == g (a 2-D strided moving operand), packing the 4
strips into one (128, 512) PSUM bank via PE column strips. A single DVE
StreamTranspose (32x32 blocks transposed in place) then yields dist for token
t = 128*m + 32*g + q at partition 32*g+q of f-tile m — i.e. plain consecutive
128-token tiles, so the x transposes use contiguous stationary slices and mm2
needs no extra data movement. This replaces 32 PE dist-transposes + PSUM
copies per supertile. Weight (stationary) APs must be 1-D free on real HW;
gpsimd cannot touch PSUM or run 3-operand TensorScalarPtr.

Sharding: batch B=16 split across 8 cores (2 per core); codes/scale replicated.
"""

import sys

sys.path.insert(0, "/opt/trn_rl_repo")
import numpy as np

import concourse.bass as bass
import concourse.bacc as bacc
import concourse.tile as tile
from concourse import mybir
from concourse.masks import make_identity

FP32 = mybir.dt.float32
BF16 = mybir.dt.bfloat16
AF = mybir.ActivationFunctionType
ALU = mybir.AluOpType
AX = mybir.AxisListType

K = 32
P = 128

B_FULL, D_FULL, H_FULL, W_FULL = 16, 512, 64, 64
N_FULL = H_FULL * W_FULL
NCORES = 8
BS = B_FULL // NCORES

ST = 2048  # tokens per supertile
NST = N_FULL // ST  # supertiles per batch
NF = ST // P  # 16 f-blocks (of 128 gathered tokens) per supertile
G = 4  # chunk groups per supertile
DT = D_FULL // P  # 4 d-tiles

# engine rotation: psx -> xt pair-copies (8 per supertile) and x2 ops (16)
COPY_ENG = ["v", "v", "a", "v", "a", "v", "a", "v"]
X2_ENG = ["a", "v", "a", "v", "a", "v", "v", "a", "v", "a", "v", "a", "a", "v", "a", "v"]
# last supertile: keep the slow Pool engine off the critical tail
COPY_ENG_LAST = ["a", "v", "a", "v", "a", "v", "a", "a"]
X2_ENG_LAST = ["v", "a"] * 8


def build(nc, bs=BS, d=D_FULL, n=N_FULL):
    x_d = nc.dram_tensor("x", (bs, DT, P, n), FP32, kind="ExternalInput").ap()
    codes_d = nc.dram_tensor("codes", (K, d), FP32, kind="ExternalInput").ap()
    scale_d = nc.dram_tensor("scale", (K, 1), FP32, kind="ExternalInput").ap()
    e_d = nc.dram_tensor("e", (bs, K, d), FP32, kind="ExternalOutput").ap()

    with tile.TileContext(nc) as tc:
        with (
            tc.tile_pool(name="const", bufs=1) as constp,
            tc.tile_pool(name="xb", bufs=3) as xbp,
            tc.tile_pool(name="xt", bufs=3) as xtp,
            tc.tile_pool(name="dist", bufs=2) as distp,
            tc.tile_pool(name="smax", bufs=2) as smaxp,
            tc.tile_pool(name="misc", bufs=4) as miscp,
            tc.tile_pool(name="tail", bufs=2) as tailp,
            tc.tile_pool(name="ps_sup", bufs=2, space="PSUM") as ps_supp,
            tc.tile_pool(name="ps_x", bufs=4, space="PSUM") as psxp,
            tc.tile_pool(name="ps_e", bufs=1, space="PSUM") as ps_ep,
            tc.tile_pool(name="ps_small", bufs=1, space="PSUM") as ps_smallp,
            tc.tile_pool(name="dstage", bufs=1, space="DRAM") as dstagep,
        ):
            # ---------------- one-time constants ----------------
            codes_sb = constp.tile([K, d], FP32)
            nc.sync.dma_start(out=codes_sb, in_=codes_d)
            scale_col = constp.tile([K, 1], FP32)
            nc.sync.dma_start(out=scale_col, in_=scale_d)
            scale_row = constp.tile([1, K], FP32)
            nc.sync.dma_start(out=scale_row, in_=scale_d.rearrange("a b -> b a"))

            ident_bf = constp.tile([P, P], BF16)
            make_identity(nc, ident_bf)

            s2_col = constp.tile([K, 1], FP32)
            nc.vector.tensor_mul(s2_col, scale_col, scale_col)
            s2_row = constp.tile([1, K], FP32)
            nc.vector.tensor_mul(s2_row, scale_row, scale_row)

            sq_codes = constp.tile([K, d], FP32)
            c2_col = constp.tile([K, 1], FP32)
            nc.scalar.activation(
                out=sq_codes, in_=codes_sb, func=AF.Square, accum_out=c2_col
            )
            # c2 as a row via a DRAM round-trip (SBUF APs cannot cross partitions)
            stage_c2 = dstagep.tile([K, 1], FP32)
            nc.sync.dma_start(out=stage_c2, in_=c2_col)
            c2_row = constp.tile([1, K], FP32)
            nc.sync.dma_start(out=c2_row, in_=stage_c2[:].rearrange("a b -> b a"))
            neg2s2_col = constp.tile([K, 1], FP32)
            nc.vector.tensor_scalar_mul(neg2s2_col, s2_col, -2.0)

            # mm1 weights w[k, d] = -2 * s2[k] * codes[k, d], transposed into
            # (P, DT*K) column blocks, bf16
            w_kn = constp.tile([K, d], BF16)
            nc.vector.tensor_scalar_mul(w_kn, codes_sb, neg2s2_col)
            w_mm1 = constp.tile([P, DT * K], BF16)
            for j in range(DT):
                psw = psxp.tile([P, 2, 512], BF16, tag="psx")
                nc.tensor.transpose(
                    psw[:, 0, :K], w_kn[:, j * P : (j + 1) * P], ident_bf[:K, :K]
                )
                nc.vector.tensor_copy(w_mm1[:, j * K : (j + 1) * K], psw[:, 0, :K])

            # s2c2 as a (1, G*K) bf16 row: the mm1 init matmul adds it to every
            # column strip while seeding the bank's has_written bits
            s2c2_row = constp.tile([1, 1, K], FP32)
            nc.vector.tensor_mul(s2c2_row, s2_row, c2_row)
            s2c2_pack = constp.tile([1, G, K], BF16)

            # s2m = s2 - max(s2), broadcast to all partitions with k on free
            s2max = constp.tile([1, 1], FP32)
            nc.vector.reduce_max(s2max, s2_row, axis=AX.X)
            s2m_row = constp.tile([1, K], FP32)
            nc.vector.tensor_scalar_sub(s2m_row, s2_row, s2max)
            stage_s2m = dstagep.tile([1, 1, K], FP32)
            nc.sync.dma_start(out=stage_s2m, in_=s2m_row)
            s2m_bc = constp.tile([P, 1, K], FP32)
            nc.sync.dma_start(out=s2m_bc, in_=stage_s2m[:].to_broadcast([P, 1, K]))

            ones_row = constp.tile([1, NF, K], BF16)
            nc.vector.memset(ones_row, 1.0)
            ones_col_f32 = constp.tile([P, 1], FP32)
            nc.vector.memset(ones_col_f32, 1.0)
            zeros_row = constp.tile([1, P], BF16)
            nc.vector.memset(zeros_row, 0.0)

            # ---------------- x loads: SWDGE cast-DMAs, finer for early tiles --
            sts = [(b, sg) for b in range(bs) for sg in range(NST)]
            xb_all = []
            for si, (b, sg) in enumerate(sts):
                xb = xbp.tile([P, DT, NF, G, K], BF16, tag="xb", name="xb")
                xb_all.append(xb)

            def emit_load(si, parts):
                b, sg = sts[si]
                mstep = NF // parts
                for gp in range(parts):
                    m0 = gp * mstep
                    t0 = sg * ST + m0 * P
                    nc.gpsimd.dma_start(
                        out=xb_all[si][:, :, m0 : m0 + mstep, :, :, ],
                        in_=x_d[b, :, :, t0 : t0 + mstep * P].transpose([1, 0, 2]),
                    )

            emit_load(0, 4)
            # issued between load gens: late enough that its input is ready,
            # early enough for supertile 0 (only gpsimd DMAs can cast)
            nc.gpsimd.dma_start(
                out=s2c2_pack, in_=s2c2_row[:].to_broadcast([1, G, K])
            )
            emit_load(1, 4)

            # ---------------- supertile pipeline ----------------
            xt_all, a_all, asum_all, x2_all, dist_all = {}, {}, {}, {}, {}
            ps_e_all, ps_cs_all = {}, {}

            def emit_softmax_part(si, nparts, h):
                nf2 = NF // nparts
                sl = slice(h * nf2, (h + 1) * nf2)
                eng = nc.vector if si == nsts - 1 else nc.gpsimd
                m1 = smaxp.tile([P, nf2, K], FP32, tag=f"m1{h % 2}")
                eng.tensor_mul(
                    m1,
                    s2m_bc[:].to_broadcast([P, nf2, K]),
                    x2_all[si][:, sl, :].to_broadcast([P, nf2, K]),
                )
                eng.tensor_add(m1, m1, dist_all[si][:, sl, :])
                pexp = smaxp.tile([P, nf2, K], FP32, tag=f"pexp{h % 2}")
                nc.scalar.activation(pexp, m1, AF.Exp)
                scol = smaxp.tile([P, nf2, 1], FP32, tag=f"scol{h % 2}")
                nc.vector.reduce_sum(scol, pexp, axis=AX.X)
                rcol = smaxp.tile([P, nf2, 1], FP32, tag=f"rcol{h % 2}")
                nc.vector.reciprocal(rcol, scol)
                nc.gpsimd.tensor_mul(
                    a_all[si][:, sl, :], pexp, rcol[:].to_broadcast([P, nf2, K])
                )

            def emit_asum(si, nparts, h):
                # f-reduction of a for the colsum matmul (fp32 accumulate)
                nf2 = NF // nparts
                sl = slice(h * nf2, (h + 1) * nf2)
                nc.vector.tensor_reduce(
                    out=asum_all[si][:, :, h : h + 1],
                    in_=a_all[si][:, sl, :].transpose([0, 2, 1]),
                    axis=AX.X,
                    op=ALU.add,
                )

            def emit_mm2_pair(pi, fp):
                pb, psg = sts[pi]
                first = psg == 0 and fp == 0
                if first:
                    ps_e_all[pb] = ps_ep.tile([K, 512], FP32, tag="e", name="pse")
                for f in (2 * fp, 2 * fp + 1):
                    nc.tensor.matmul(
                        ps_e_all[pb],
                        a_all[pi][:, f, :],
                        xt_all[pi][:, f, :],
                        start=(first and f == 0),
                        stop=(psg == NST - 1 and f == NF - 1),
                        skip_group_check=True,
                    )

            def emit_cs(pi):
                pb, psg = sts[pi]
                nparts = nparts_all[pi]
                if psg == 0:
                    ps_cs_all[pb] = ps_smallp.tile([K, 1], FP32, tag="cs", name="pscs")
                for h in range(nparts):
                    nc.tensor.matmul(
                        ps_cs_all[pb],
                        asum_all[pi][:, :, h : h + 1],
                        ones_col_f32,
                        start=(psg == 0 and h == 0),
                        stop=(psg == NST - 1 and h == nparts - 1),
                        skip_group_check=True,
                    )

            negcs_all = {}

            def emit_negcs(pi):
                pb, psg = sts[pi]
                if psg == NST - 1:
                    negcs_all[pb] = tailp.tile([K, 1], FP32, tag="negcs", name="negcs")
                    nc.scalar.mul(negcs_all[pb], ps_cs_all[pb], -1.0)

            def emit_tail(pi):
                pb, psg = sts[pi]
                if psg == NST - 1:
                    # tail: e = e1 - cs * codes, read straight out of PSUM
                    negcs = negcs_all[pb]
                    e_sb = tailp.tile([K, d], FP32, tag="esb")
                    nc.vector.scalar_tensor_tensor(
                        out=e_sb,
                        in0=codes_sb,
                        scalar=negcs,
                        in1=ps_e_all[pb],
                        op0=ALU.mult,
                        op1=ALU.add,
                    )
                    nc.sync.dma_start(out=e_d[pb], in_=e_sb)

            def emit_mm1(si, ghalf):
                """Half of si's mm1 (column groups 2*ghalf, 2*ghalf+1)."""
                xb = xb_all[si]
                if ghalf == 0:
                    ps_sup = ps_supp.tile([P, NF, K], FP32, tag="sup", name="sup")
                    ps_sup_all[si] = ps_sup
                    nc.tensor.matmul(
                        ps_sup,
                        zeros_row,
                        ones_row,
                        start=True,
                        stop=False,
                        skip_group_check=True,
                    )
                mranges = [(m, m + 4) for m in range(0, NF, 4)]
                for m0, m1 in mranges:
                    for g in (2 * ghalf, 2 * ghalf + 1):
                        for j in range(DT):
                            nc.tensor.matmul(
                                ps_sup_all[si][32 * g : 32 * g + 32, m0:m1, :],
                                w_mm1[:, j * K : (j + 1) * K],
                                xb[:, j, m0:m1, g, :],
                                start=False,
                                stop=False,
                                tile_position=(0, 32 * g),
                                skip_group_check=True,
                            )
                if ghalf == 1 and si > 0:
                    emit_s2c2_edge(si)

            def emit_s2c2_edge(si):
                # s2c2 bias for every strip; kept off the critical setup
                # path by running it last (also carries the stop)
                nc.tensor.matmul(
                    ps_sup_all[si],
                    s2c2_pack,
                    ones_row,
                    start=False,
                    stop=True,
                    skip_group_check=True,
                )

            def emit_x2(si, f):
                x2e = (X2_ENG_LAST if si == nsts - 1 else X2_ENG)[f]
                src = xt_all[si][:, f, :]
                sqs = miscp.tile([P, 512], BF16, tag="sqs")
                if x2e == "a":
                    nc.scalar.activation(
                        out=sqs,
                        in_=src,
                        func=AF.Square,
                        accum_out=x2_all[si][:, f, :],
                    )
                else:
                    nc.vector.scalar_tensor_tensor(
                        out=sqs,
                        in0=src,
                        scalar=1.0,
                        in1=src,
                        op0=ALU.mult,
                        op1=ALU.mult,
                        accum_out=x2_all[si][:, f, :],
                    )

            ps_sup_all = {}
            nparts_all = {}
            nsts = len(sts)

            def emit_prepare_next(pi):
                fp = prep_ctr[pi]
                if fp >= NF // 2:
                    return
                if fp == 0:
                    emit_prepare_alloc(pi)
                prep_ctr[pi] += 1
                emit_prepare_fp(pi, fp)

            def emit_prepare_fp(pi, fp):
                """Transposes + psx copy + x2 for f-pair fp of supertile pi."""
                xb = xb_all[pi]
                psx = psxp.tile([P, 2, 512], BF16, tag="psx")
                for tt in range(2):
                    f = 2 * fp + tt
                    for j in range(DT):
                        nc.tensor.transpose(
                            psx[:, tt, j * P : (j + 1) * P],
                            xb[:, j, f, :, :],
                            ident_bf,
                        )
                ce = COPY_ENG[fp]
                xt_pair = xt_all[pi][:, 2 * fp : 2 * fp + 2, :]
                if ce == "v":
                    nc.vector.tensor_copy(xt_pair, psx)
                elif ce == "a":
                    nc.scalar.copy(xt_pair, psx)
                else:
                    nc.gpsimd.tensor_copy(xt_pair, psx)
                if fp > 0:
                    # x2 for the previous pair, so it never delays a copy
                    emit_x2(pi, 2 * fp - 2)
                    emit_x2(pi, 2 * fp - 1)

            def emit_prepare_alloc(pi):
                xt_all[pi] = xtp.tile([P, NF, 512], BF16, tag="xt", name="xt")
                x2_all[pi] = miscp.tile([P, NF, 1], FP32, tag="x2", name="x2")
                a_all[pi] = smaxp.tile([P, NF, K], BF16, tag="a", name="a")
                nparts_all[pi] = 2
                asum_all[pi] = smaxp.tile([P, K, 2], FP32, tag="asum", name="asum")

            def emit_streamT(pi):
                dist = distp.tile([P, NF, K], FP32, tag="dist")
                dist_all[pi] = dist
                nc.vector.transpose(dist, ps_sup_all[pi])

            pair_ctr = {si: 0 for si in range(nsts)}
            prep_ctr = {si: 0 for si in range(nsts)}
            x2_done = {si: False for si in range(nsts)}
            sm_done = {si: 0 for si in range(nsts)}  # softmax halves emitted

            def emit_mm2_n(pi, n):
                for _ in range(n):
                    fp = pair_ctr[pi]
                    if fp >= NF // 2:
                        return
                    pair_ctr[pi] += 1
                    emit_mm2_pair(pi, fp)

            def emit_sm(si):
                emit_softmax_part(si, 2, sm_done[si])
                sm_done[si] += 1

            # ---------------- prologue: fully prepare supertile 0 ------------
            emit_mm1(0, 0)
            emit_mm1(0, 1)
            for fp in range(NF // 2):
                emit_prepare_next(0)
                if fp == 3:
                    emit_mm1(1, 0)
                elif fp == 4:
                    emit_s2c2_edge(0)
                    emit_streamT(0)
                elif fp == 5:
                    emit_sm(0)
            emit_mm1(1, 1)
            x2_done[0] = True
            emit_x2(0, NF - 2)
            emit_x2(0, NF - 1)
            emit_mm2_n(0, 2)

            # ---------------- main blocks: softmax+mm2(si), prepare(si+1) ----
            for si in range(nsts):
                nxt = si + 1 if si + 1 < nsts else None
                same_batch_nxt = nxt is not None and sts[nxt][0] == sts[si][0]
                while sm_done[si] < 2 and (sm_done[si] == 0 or si == nsts - 1):
                    emit_sm(si)
                pre_done = sm_done[si] == 2 and si == nsts - 1
                if pre_done:
                    emit_asum(si, 2, 0)
                    emit_asum(si, 2, 1)
                if nxt is not None:
                    emit_streamT(nxt)
                for fp in range(NF // 2):
                    if nxt is not None:
                        emit_prepare_next(nxt)
                        if prep_ctr[nxt] == NF // 2 and not x2_done[nxt]:
                            x2_done[nxt] = True
                            emit_x2(nxt, NF - 2)
                            emit_x2(nxt, NF - 1)
                            if same_batch_nxt and nxt == nsts - 1:
                                emit_sm(nxt)
                    emit_mm2_n(si, 1)
                    if fp == 1 and si + 2 < nsts:
                        emit_load(si + 2, 4)
                    if fp == 1 and sm_done[si] < 2:
                        emit_sm(si)
                    if fp == 2 and not pre_done:
                        emit_asum(si, 2, 0)
                    if fp == 2 and pre_done:
                        emit_cs(si)
                        emit_negcs(si)
                    if fp == 5:
                        if sm_done[si] == 2 and not pre_done:
                            emit_asum(si, 2, 1)
                        if si + 2 < nsts:
                            emit_mm1(si + 2, 0)
                        if same_batch_nxt:
                            emit_sm(nxt)
                    if fp == 6 and sm_done[si] == 2 and not pre_done:
                        emit_cs(si)
                        emit_negcs(si)
                    elif fp == 7 and si + 2 < nsts:
                        emit_mm1(si + 2, 1)
                if si + 2 == nsts - 1:
                    emit_prepare_next(nsts - 1)
                    emit_prepare_next(nsts - 1)
                if nxt is not None and not x2_done[nxt] and prep_ctr[nxt] == NF // 2:
                    x2_done[nxt] = True
                    emit_x2(nxt, NF - 2)
                    emit_x2(nxt, NF - 1)
                    if same_batch_nxt and nxt == nsts - 1:
                        emit_sm(nxt)
                emit_mm2_n(si, NF // 2)  # any pairs not yet emitted
                if same_batch_nxt:
                    emit_mm2_n(nxt, 2)
                emit_tail(si)


_CACHE = {}


def _get_compiled():
    if "nc" not in _CACHE:
        nc = bacc.Bacc(
            "TRN2",
            target_bir_lowering=False,
            debug=False,
            dynamic_dma_scratch_size=98304,
        )
        build(nc)
        nc.compile()
        _CACHE["nc"] = nc
    return _CACHE["nc"]


def kernel(x, codes, scale):
    from concourse import bass_utils

    b_total = x.shape[0]
    bs = b_total // NCORES
    xr = np.ascontiguousarray(x.reshape(b_total, DT, P, -1), dtype=np.float32)
    codes_c = np.ascontiguousarray(codes, dtype=np.float32)
    scale_c = np.ascontiguousarray(scale, dtype=np.float32).reshape(K, 1)

    nc = _get_compiled()
    in_maps = [
        {"x": xr[i * bs : (i + 1) * bs], "codes": codes_c, "scale": scale_c}
        for i in range(NCORES)
    ]
    res = bass_utils.run_bass_kernel_spmd(nc, in_maps, core_ids=list(range(NCORES)))
    e = np.concatenate([r["e"] for r in res.results], axis=0)
    return e.astype(np.float32)
